# revision 36
# baseline (speedup 1.0000x reference)
"""Bidirectional ConvLSTM Trainium2 kernel v4 (8-core SPMD, bf16 datapath).

Sharding: core 2b = forward dir of batch b, core 2b+1 = backward (host
feeds time-reversed x). Per step, bank-major conv (8 PSUM banks, 9-tap
accumulate per bank) with drain chasing: each finished bank is copied to
SBUF (ACT) and bn_stats'd (DVE) while later banks still matmul. Gate halves
are [g;i] then [f;o] so the final activation is a single 128-partition
sigmoid straight from PSUM. GN finalize avoids PSUM/PE entirely: one gpsimd
partition_all_reduce over mask-folded group sums + a 2-iteration Newton
rsqrt on DVE. The state update is 1024-chunk-chased so the next step's conv
starts on early h rows; each step's fusion matmul (wfu^T @ h) is deferred
one step so it fills the PE idle window during the h critical chain. Fusion
partials (bf16) are pair-exchanged with 2-step AllGathers fired inside the
loop (the last exchange is split so only a half-chunk AG is exposed); the
redundant BN tail processes chunks in exchange-availability order inside
the loop pool scope, with only the BN-stats AllReduce + ReLU + output DMA
after the loop.
"""

import numpy as np
import concourse.bass as bass
import concourse.bacc as bacc
import concourse.bass_isa as bass_isa
import concourse.mybir as mybir
import concourse.tile as tile
from concourse.bass_utils import run_bass_kernel_spmd

fp32 = mybir.dt.float32
bf16 = mybir.dt.bfloat16
i32 = mybir.dt.int32
Alu = mybir.AluOpType
Act = mybir.ActivationFunctionType

T = 16
HID = 64
S = 4096
EPS = 1e-5
N_CORES = 8
CORE_IDS = list(range(N_CORES))
MAGIC = 0x5F3759DF
LDW_PROBE = False
COARSE_CHAIN = True
STATE_BOUNDS = [0, 1024, 2048, 3072, 4096]


def emit_rsqrt(nc, pool, x_ap, out_ap, iters=2):
    P = x_ap.shape[0]
    yi = pool.tile([P, 1], i32, tag=f"rsq_i{P}")
    t = pool.tile([P, 1], fp32, tag=f"rsq_t{P}")
    nc.vector.tensor_scalar(yi[:], x_ap.bitcast(i32), 1, None,
                            Alu.logical_shift_right)
    nc.vector.tensor_scalar(yi[:], yi[:], -1, MAGIC, Alu.mult, Alu.add)
    y = yi[:].bitcast(fp32)
    for i in range(iters):
        nc.vector.tensor_mul(t[:], y, y)
        nc.vector.tensor_mul(t[:], t[:], x_ap)
        nc.vector.tensor_scalar(t[:], t[:], -0.5, 1.5, Alu.mult, Alu.add)
        nc.vector.tensor_mul(out_ap if i == iters - 1 else y, y, t[:])


def emit_fusion(nc, wp, pgpool, wfu_r, src, dst, ags, tail, sim):
    """Fusion partial p_s = wfu^T @ h(s); h(s) lives in `src` (p64-127).
    dst: DRAM AP [64, S] to write; ags: list of (ins, outs) AllGathers to
    fire afterwards."""
    psb = wp.tile([64, S], bf16, tag="psb")
    for bk in range(8):
        pf = pgpool.tile([64, 512], fp32, tag="pg")
        nc.tensor.matmul(pf[:], wfu_r[:],
                         src[:, 1 + bk * 8:1 + bk * 8 + 8, 2:66],
                         start=True, stop=True)
        if bk % 2 == 0:
            nc.scalar.copy(psb[:, bk * 512:(bk + 1) * 512], pf[:])
        else:
            nc.vector.tensor_copy(psb[:, bk * 512:(bk + 1) * 512], pf[:])
    nc.sync.dma_start(dst[:, 0:2048], psb[:, 0:2048])
    nc.sync.dma_start(dst[:, 2048:4096], psb[:, 2048:4096])
    if tail and not sim:
        for ins, outs in ags:
            nc.gpsimd.collective_compute(
                "AllGather", Alu.bypass,
                replica_groups=[[0, 1], [2, 3], [4, 5], [6, 7]],
                ins=[ins], outs=[outs],
            )


def build_program(nsteps=T, reps=1, sim=False, tail=True):
    nc = bacc.Bacc("TRN2", target_bir_lowering=False, debug=False,
                   num_devices=1 if sim else N_CORES)

    xs = nc.dram_tensor("xs", [nsteps, 64, 64, 64], bf16,
                        kind="ExternalInput").ap()
    wconv = nc.dram_tensor("wconv", [9, 2, 128, 128], bf16,
                           kind="ExternalInput").ap()
    wfu = nc.dram_tensor("wfu", [128, 64], bf16, kind="ExternalInput").ap()
    gnw = nc.dram_tensor("gnw", [2, 128], fp32, kind="ExternalInput").ap()
    gnb = nc.dram_tensor("gnb", [2, 128], fp32, kind="ExternalInput").ap()
    bnw = nc.dram_tensor("bnw", [64, 1], fp32, kind="ExternalInput").ap()
    bnb = nc.dram_tensor("bnb", [64, 1], fp32, kind="ExternalInput").ap()
    bind = nc.dram_tensor("bind", [128, 64], fp32, kind="ExternalInput").ap()
    bindT = nc.dram_tensor("bindT", [64, 128], fp32,
                           kind="ExternalInput").ap()
    out = nc.dram_tensor("out", [nsteps, 64, S], fp32,
                         kind="ExternalOutput").ap()

    nch2 = nsteps // 2
    psend = [nc.dram_tensor(f"psend{k}", [2, 64, S], bf16)
             for k in range(nch2 - 1)]
    pgath = [nc.dram_tensor(f"pgath{k}", [2, 2, 64, S], bf16)
             for k in range(nch2 - 1)]
    p14 = nc.dram_tensor("p14", [64, S], bf16)
    p15 = nc.dram_tensor("p15", [64, S], bf16)
    g14 = nc.dram_tensor("g14", [2, 64, S], bf16)
    g15 = nc.dram_tensor("g15", [2, 64, S], bf16)
    bnps = nc.dram_tensor("bnps", [64, 2], fp32)
    bnpr = nc.dram_tensor("bnpr", [64, 2], fp32,
                          **({} if sim else {"addr_space": "Shared"}))

    with tile.TileContext(nc) as tc:
      with tc.tile_pool(name="const", bufs=1) as cp:
        bind_r = cp.tile([128, 64], fp32, tag="bindr")
        nc.sync.dma_start(bind_r[:], bind)
        bindT_r = cp.tile([64, 128], fp32, tag="bindTr")
        nc.sync.dma_start(bindT_r[:], bindT)
        bnw_sb = cp.tile([64, 1], fp32, tag="bnw")
        nc.sync.dma_start(bnw_sb[:], bnw)
        bnb_sb = cp.tile([64, 1], fp32, tag="bnb")
        nc.sync.dma_start(bnb_sb[:], bnb)
        # group-fold masks for partition_all_reduce
        mkA = cp.tile([128, 2], fp32, tag="mkA")
        nc.vector.memset(mkA[:], 0.0)
        nc.vector.memset(mkA[0:64, :], 1.0)
        mkB = cp.tile([128, 2], fp32, tag="mkB")
        nc.vector.memset(mkB[:], 0.0)
        nc.vector.memset(mkB[64:128, :], 1.0)

        for rep in range(reps):
          with (
            tc.tile_pool(name=f"persist{rep}", bufs=1) as pp,
            tc.tile_pool(name=f"work{rep}", bufs=2) as wp,
            tc.tile_pool(name=f"pg{rep}", bufs=8, space="PSUM") as pgpool,
          ):
            # ---- prologue ----
            wr = pp.tile([128, 18 * 128], bf16, tag="wr")
            nc.sync.dma_start(
                wr[:].rearrange("k (t h m) -> k t h m", t=9, h=2),
                wconv.rearrange("t h k m -> k t h m"),
            )
            wfu_r = pp.tile([128, 64], bf16, tag="wfur")
            nc.sync.dma_start(wfu_r[:], wfu)
            gnw_sb = pp.tile([128, 2], fp32, tag="gnw")
            nc.sync.dma_start(gnw_sb[:], gnw.rearrange("h p -> p h"))
            gnb_sb = pp.tile([128, 2], fp32, tag="gnb")
            nc.sync.dma_start(gnb_sb[:], gnb.rearrange("h p -> p h"))

            # persistent state
            inp0 = pp.tile([128, 66, 68], bf16, tag="inp0")
            inp1 = pp.tile([128, 66, 68], bf16, tag="inp1")
            nc.vector.memset(inp0[:], 0.0)
            nc.vector.memset(inp1[:], 0.0)
            inps = [inp0, inp1]
            gi = pp.tile([128, S], bf16, tag="gi")    # g(0:64) i(64:128)
            fo = pp.tile([128, S], bf16, tag="fo")    # f(0:64) o(64:128)
            pA = pp.tile([64, S], bf16, tag="pA")     # g*i
            cC = pp.tile([64, S], bf16, tag="cC")     # cell state
            nc.vector.memset(cC[:], 0.0)

            # x(0)
            nc.sync.dma_start(inp0[0:64, 1:65, 2:66], xs[0])

            for t in range(nsteps):
                cur = inps[t % 2]
                nxt = inps[(t + 1) % 2]
                if t < nsteps - 1:
                    nc.sync.dma_start(nxt[0:64, 1:65, 2:66], xs[t + 1])

                svs, bvs = [], []
                pgs1 = []
                for half in range(2):
                    st_t = wp.tile([128, 8, 6], fp32, tag=f"st{half}")
                    for bk in range(8):
                        pg = pgpool.tile([128, 512], fp32, tag="pg")
                        y0 = bk * 8
                        for tap in range(9):
                            dy, dx = tap // 3, tap % 3
                            _ti = 0 if LDW_PROBE else tap
                            lhsT = wr[:, (_ti * 2 + half) * 128:
                                      (_ti * 2 + half + 1) * 128]
                            nc.tensor.matmul(
                                pg[:],
                                lhsT,
                                cur[:, y0 + dy:y0 + dy + 8, dx + 1:dx + 65],
                                start=(tap == 0), stop=(tap == 8),
                            )
                        if half == 0:
                            # copy first; stats read the bf16 copy (2x DVE)
                            nc.scalar.copy(gi[:, bk * 512:(bk + 1) * 512],
                                           pg[:])
                            nc.vector.bn_stats(
                                st_t[:, bk, :],
                                gi[:, bk * 512:(bk + 1) * 512])
                        else:
                            nc.vector.bn_stats(st_t[:, bk, :], pg[:])
                            pgs1.append(pg)

                    # ---- GN finalize for this half ----
                    aggr = wp.tile([128, 2], fp32, tag=f"aggr{half}")
                    nc.vector.bn_aggr(aggr[:], st_t[:])
                    e2 = wp.tile([128, 2], fp32, tag=f"e2{half}")
                    nc.vector.tensor_mul(e2[:, 0:1], aggr[:, 0:1],
                                         aggr[:, 0:1])
                    nc.vector.tensor_add(e2[:, 1:2], aggr[:, 1:2],
                                         e2[:, 0:1])
                    nc.vector.tensor_copy(e2[:, 0:1], aggr[:, 0:1])
                    s4 = wp.tile([128, 4], fp32, tag=f"s4{half}")
                    nc.vector.tensor_mul(s4[:, 0:2], e2[:], mkA[:])
                    nc.vector.tensor_mul(s4[:, 2:4], e2[:], mkB[:])
                    p4 = wp.tile([128, 4], fp32, tag=f"p4{half}")
                    nc.gpsimd.partition_all_reduce(p4[:], s4[:], 128,
                                                   bass_isa.ReduceOp.add)
                    sel = wp.tile([128, 2], fp32, tag=f"sel{half}")
                    nc.vector.tensor_mul(sel[:], p4[:, 0:2], mkA[:])
                    t2 = wp.tile([128, 2], fp32, tag=f"t2{half}")
                    nc.vector.tensor_mul(t2[:], p4[:, 2:4], mkB[:])
                    nc.vector.tensor_add(sel[:], sel[:], t2[:])
                    mE = wp.tile([128, 2], fp32, tag=f"mE{half}")
                    nc.vector.tensor_scalar_mul(mE[:], sel[:], 1.0 / 64.0)
                    mu2 = wp.tile([128, 1], fp32, tag=f"mu2{half}")
                    nc.vector.tensor_mul(mu2[:], mE[:, 0:1], mE[:, 0:1])
                    varx = wp.tile([128, 1], fp32, tag=f"varx{half}")
                    nc.vector.scalar_tensor_tensor(varx[:], mE[:, 1:2], EPS,
                                                   mu2[:], Alu.add,
                                                   Alu.subtract)
                    rstd = wp.tile([128, 1], fp32, tag=f"rstd{half}")
                    emit_rsqrt(nc, wp, varx[:], rstd[:])
                    sv = wp.tile([128, 1], fp32, tag=f"sv{half}")
                    nc.vector.tensor_mul(sv[:], rstd[:],
                                         gnw_sb[:, half:half + 1])
                    tv = wp.tile([128, 1], fp32, tag=f"tv{half}")
                    nc.vector.tensor_mul(tv[:], mE[:, 0:1], sv[:])
                    bv = wp.tile([128, 1], fp32, tag=f"bv{half}")
                    nc.vector.tensor_sub(bv[:], gnb_sb[:, half:half + 1],
                                         tv[:])
                    svs.append(sv)
                    bvs.append(bv)

                    if half == 0:
                        # acts from SBUF copies during half1 conv
                        for ck in range(4):
                            sl = slice(ck * 1024, (ck + 1) * 1024)
                            nc.scalar.activation(gi[0:64, sl], gi[0:64, sl],
                                                 Act.Tanh,
                                                 bias=bvs[0][0:64, :],
                                                 scale=svs[0][0:64, :])
                            nc.scalar.activation(fo[0:64, sl],
                                                 gi[64:128, sl], Act.Sigmoid,
                                                 bias=bvs[0][64:128, :],
                                                 scale=svs[0][64:128, :])
                            nc.vector.tensor_mul(pA[:, sl], gi[0:64, sl],
                                                 fo[0:64, sl])

                # half1: sigmoid(f,o) straight from PSUM, then chunk-chased
                # state update + h write (emission interleaved per chunk --
                # ACT/DVE queues are strict FIFO; leading chunks are 512
                # wide to shorten the h critical chain)
                bounds = STATE_BOUNDS
                for ci in range(len(bounds) - 1):
                    lo, hi = bounds[ci], bounds[ci + 1]
                    sl = slice(lo, hi)
                    for bk in range(lo // 512, hi // 512):
                        bsl = slice(bk * 512, (bk + 1) * 512)
                        nc.scalar.activation(fo[:, bsl], pgs1[bk][:],
                                             Act.Sigmoid,
                                             bias=bvs[1][:], scale=svs[1][:])
                    nc.vector.tensor_mul(cC[:, sl], cC[:, sl], fo[0:64, sl])
                    nc.vector.tensor_add(cC[:, sl], cC[:, sl], pA[:, sl])
                    # tanh(c') into gi's dead rows (base-64 aligned for hmul)
                    nc.scalar.activation(gi[64:128, sl], cC[:, sl], Act.Tanh)
                    r0, nr = lo // 64, (hi - lo) // 64
                    nc.vector.tensor_mul(
                        nxt[64:128, 1 + r0:1 + r0 + nr, 2:66],
                        fo[64:128, sl].rearrange("p (a b) -> p a b", a=nr),
                        gi[64:128, sl].rearrange("p (a b) -> p a b", a=nr),
                    )

                # deferred fusion for h(t-1): dependency-free PE work that
                # fills the idle window while this step's h chain runs
                if t > 0:
                    s = t - 1
                    if s < 14:
                        dst = psend[s // 2][s % 2]
                        ags = ([(psend[s // 2][:], pgath[s // 2][:])]
                               if s % 2 == 1 else [])
                    else:
                        dst = p14[:]
                        ags = [(p14[:], g14[:])]
                    emit_fusion(nc, wp, pgpool, wfu_r, cur, dst, ags,
                                tail, sim)
            # post-loop: fusion for the last step's h
            emit_fusion(nc, wp, pgpool, wfu_r, inps[nsteps % 2], p15[:],
                        [(p15[:], g15[:])], tail, sim)

          # ---- fusion tail (redundant on both cores of a pair) ----
          # runs inside the loop pool scope so chunk adds/stats overlap the
          # last steps of the time loop as AllGathers land; chunks are
          # processed in exchange-availability order.
            if tail:
              nu = nsteps // 2
              F = pp.tile([128, nu, S], bf16, tag="F")
              st_t = pp.tile([128, nu, 8, 6], fp32, tag="stT")
              for k in [3, 4, 2, 5, 1, 6, 7, 0]:
                tB = wp.tile([128, S], bf16, tag="tB")
                if k == 7:
                    nc.sync.dma_start(F[0:64, k, :], g14[0])
                    nc.sync.dma_start(F[64:128, k, :], g15[0])
                else:
                    nc.sync.dma_start(F[0:64, k, :], pgath[k][0, 0])
                    nc.sync.dma_start(F[64:128, k, :], pgath[k][0, 1])
                if k == 0:
                    nc.sync.dma_start(tB[0:64, :], g15[1])
                    nc.sync.dma_start(tB[64:128, :], g14[1])
                else:
                    nc.sync.dma_start(tB[0:64, :], pgath[nu - 1 - k][1, 1])
                    nc.sync.dma_start(tB[64:128, :], pgath[nu - 1 - k][1, 0])
                nc.vector.tensor_add(F[:, k, :], F[:, k, :], tB[:])
                for q in range(8):
                    nc.vector.bn_stats(st_t[:, k, q, :],
                                       F[:, k, q * 512:(q + 1) * 512])

              aggr = wp.tile([128, 2], fp32, tag="taggr")
              nc.vector.bn_aggr(aggr[:], st_t[:])
              s2 = wp.tile([128, 2], fp32, tag="ts2")
              nc.vector.tensor_copy(s2[:, 0:1], aggr[:, 0:1])
              t128 = wp.tile([128, 1], fp32, tag="t128")
              nc.vector.tensor_mul(t128[:], aggr[:, 0:1], aggr[:, 0:1])
              nc.vector.tensor_add(s2[:, 1:2], aggr[:, 1:2], t128[:])
              smg = pgpool.tile([64, 512], fp32, tag="pg")
              nc.tensor.matmul(smg[:, 0:2], bind_r[:], s2[:],
                               start=True, stop=True)
              bsb = wp.tile([64, 2], fp32, tag="bsb")
              nc.scalar.copy(bsb[:], smg[:, 0:2])
              nc.sync.dma_start(bnps[:], bsb[:])
              if not sim:
                nc.gpsimd.collective_compute(
                    "AllReduce", Alu.add,
                    replica_groups=[CORE_IDS],
                    ins=[bnps[:]], outs=[bnpr[:]],
                )
              s16 = wp.tile([64, 2], fp32, tag="s16")
              nc.sync.dma_start(s16[:], bnpr[:])
              mE = wp.tile([64, 2], fp32, tag="mEt")
              nc.vector.tensor_scalar_mul(mE[:], s16[:], 1.0 / 16.0)
              mu2 = wp.tile([64, 1], fp32, tag="tmu2")
              nc.vector.tensor_mul(mu2[:], mE[:, 0:1], mE[:, 0:1])
              varx = wp.tile([64, 1], fp32, tag="tvarx")
              nc.vector.scalar_tensor_tensor(varx[:], mE[:, 1:2], EPS,
                                             mu2[:], Alu.add, Alu.subtract)
              rstd = wp.tile([64, 1], fp32, tag="trstd")
              emit_rsqrt(nc, wp, varx[:], rstd[:], iters=3)
              brhs = wp.tile([64, 2], fp32, tag="tbrhs")
              nc.vector.tensor_mul(brhs[:, 0:1], bnw_sb[:], rstd[:])
              tv = wp.tile([64, 1], fp32, tag="ttv")
              nc.vector.tensor_mul(tv[:], mE[:, 0:1], brhs[:, 0:1])
              nc.vector.tensor_sub(brhs[:, 1:2], bnb_sb[:], tv[:])
              smb = pgpool.tile([128, 512], fp32, tag="pg")
              nc.tensor.matmul(smb[:, 0:2], bindT_r[:], brhs[:],
                               start=True, stop=True)
              svec = wp.tile([128, 1], fp32, tag="tsvec")
              nc.vector.tensor_copy(svec[:], smb[:, 0:1])
              bvec = wp.tile([128, 1], fp32, tag="tbvec")
              nc.vector.tensor_copy(bvec[:], smb[:, 1:2])

              for u in range(nu):
                for cc in range(2):
                    cols = slice(cc * 2048, (cc + 1) * 2048)
                    fo2 = wp.tile([128, 2048], fp32, tag="fo2")
                    nc.scalar.activation(fo2[:], F[:, u, cols], Act.Relu,
                                         bias=bvec[:], scale=svec[:])
                    nc.sync.dma_start(out[2 * u, :, cols], fo2[0:64, :])
                    nc.sync.dma_start(out[2 * u + 1, :, cols],
                                      fo2[64:128, :])

    nc.compile()
    return nc


def make_in_maps(x, Wf, gnf_w, gnf_b, Wb, gnb_w, gnb_b, Wfu, bn_w, bn_b,
                 nsteps=T):
    import ml_dtypes
    # gate rows: half0 = [g; i], half1 = [f; o]
    perm = np.concatenate([np.arange(192, 256), np.arange(0, 64),
                           np.arange(64, 128), np.arange(128, 192)])
    bind_m = np.zeros((128, 64), np.float32)
    for c in range(64):
        bind_m[c, c] = 1.0
        bind_m[c + 64, c] = 1.0
    bindT_m = np.ascontiguousarray(bind_m.T)
    Wfu2 = np.asarray(Wfu)[:, :, 0, 0]
    bft = ml_dtypes.bfloat16

    in_maps = []
    for core in range(N_CORES):
        b = core // 2
        fwd = core % 2 == 0
        xb = np.asarray(x)[b].reshape(-1, 64, 64, 64)[:nsteps]
        if not fwd:
            xb = xb[::-1]
        Wd = np.asarray(Wf if fwd else Wb)[perm]
        gw = np.asarray(gnf_w if fwd else gnb_w)[perm]
        gb = np.asarray(gnf_b if fwd else gnb_b)[perm]
        wconv_m = np.empty((9, 2, 128, 128), np.float32)
        for tap in range(9):
            dy, dx = tap // 3, tap % 3
            for half in range(2):
                wconv_m[tap, half] = \
                    Wd[half * 128:(half + 1) * 128, :, dy, dx].T
        wfu_m = np.zeros((128, 64), np.float32)
        wfu_m[64:128, :] = (Wfu2[:, 0:64] if fwd else Wfu2[:, 64:128]).T
        in_maps.append({
            "xs": np.ascontiguousarray(xb).astype(bft),
            "wconv": wconv_m.astype(bft),
            "wfu": wfu_m.astype(bft),
            "gnw": np.ascontiguousarray(gw.reshape(2, 128)),
            "gnb": np.ascontiguousarray(gb.reshape(2, 128)),
            "bnw": np.asarray(bn_w, np.float32).reshape(64, 1).copy(),
            "bnb": np.asarray(bn_b, np.float32).reshape(64, 1).copy(),
            "bind": bind_m,
            "bindT": bindT_m,
        })
    return in_maps


_cached_nc = None


def kernel(x, Wf, gnf_w, gnf_b, Wb, gnb_w, gnb_b, Wfu, bn_w, bn_b):
    global _cached_nc
    if _cached_nc is None:
        _cached_nc = build_program(T)
    nc = _cached_nc
    in_maps = make_in_maps(x, Wf, gnf_w, gnf_b, Wb, gnb_w, gnb_b, Wfu,
                           bn_w, bn_b)
    res = run_bass_kernel_spmd(nc, in_maps, CORE_IDS)
    outs = [res.results[2 * b]["out"].reshape(T, HID, 64, 64)
            for b in range(4)]
    return np.ascontiguousarray(np.stack(outs).astype(np.float32))
